# revision 4
# baseline (speedup 1.0000x reference)
"""GCN layer on 8 Trainium2 NeuronCores.

out = relu(D^{-1/2} (A+I) D^{-1/2} x W^T + b),  N=8192, D=512, A symmetric binary.

Sharding (1-D graph partition, rank c owns nodes [c*1024, (c+1)*1024)):
  - A+I is symmetric, so the row-block (A+I)[own, :] each core must aggregate
    equals the column slab (A+I)[:, own] transposed: each core is fed the
    natural column slab, which is exactly the [K, N] layout the PE wants.
  - All normalization is folded on the host (deg is a cheap host reduction):
      slab' = (A+I)[:, own] * d_own^{-1/2}[col]   (bf16)
      y     = d^{-1/2}[:, None] * x               (bf16, replicated)
    so the device does exactly two matmuls and a relu:
      hT[feat, own] = y^T @ slab'   (contract over all 8192 nodes)
      out[own, :]   = relu(hT^T @ W^T + b)
    No collectives, no cast-DMAs, no on-device degree pass.
  - Streams: slab chunks on the SP HWDGE queue, y/wt on the ACT HWDGE queue,
    rotating tile-pool buffers so DMA stays ahead of the PE.
"""

import numpy as np

N = 8192
D = 512
NCORES = 8
B = N // NCORES          # 1024 nodes per core
P = 128
KT = N // P              # 64 k-tiles of 128 rows
SCH = 8                  # slab chunks (8 k-tiles each)
SKPC = KT // SCH         # k-tiles per slab chunk
YCH = 4                  # y chunks (16 k-tiles each)
YKPC = KT // YCH

_cache = {}


def _build(with_bias: bool, ar_chunks: int = 1, reps: int = 1,
           serialize_reps: bool = False, skip_collectives: bool = False,
           num_devices: int = NCORES, mm_n1024: bool = False):
    import concourse.tile as tile
    from concourse import bacc, mybir

    f32 = mybir.dt.float32
    bf16 = mybir.dt.bfloat16

    nc = bacc.Bacc("TRN2", target_bir_lowering=False, debug=False,
                   num_devices=num_devices)

    slab_d = nc.dram_tensor("slab", [N, B], bf16, kind="ExternalInput").ap()
    y_d = nc.dram_tensor("y", [N, D], bf16, kind="ExternalInput").ap()
    wt_d = nc.dram_tensor("wt", [D, D], bf16, kind="ExternalInput").ap()
    if with_bias:
        bb_d = nc.dram_tensor("bb", [P, D], f32, kind="ExternalInput").ap()
    out_d = nc.dram_tensor("out", [B, D], f32, kind="ExternalOutput").ap()
    out_r = out_d.rearrange("(m p) f -> p m f", p=P)

    with tile.TileContext(nc) as tc:
        with tc.tile_pool(name="slab", bufs=1) as slab_pool, \
             tc.tile_pool(name="y", bufs=1) as y_pool, \
             tc.tile_pool(name="small", bufs=1) as small, \
             tc.tile_pool(name="osb", bufs=1) as osb_pool, \
             tc.tile_pool(name="psum", bufs=1, space="PSUM") as psum_pool:
          for _rep in range(reps):
            # ---- input streams (HWDGE: slab on SP, y/wt on ACT) ----
            slab_sb = []
            for ch in range(SCH):
                t = slab_pool.tile([P, SKPC, B], bf16, name=f"slab{ch}",
                                   tag=f"sl{ch % 4}")
                src = slab_d[ch * (SKPC * P):(ch + 1) * (SKPC * P), :]
                nc.sync.dma_start(t[:], src.rearrange("(n p) f -> p n f", p=P))
                slab_sb.append(t)
            wt_sb = small.tile([P, D // P, D], bf16, name="wt_sb", tag="wt",
                               bufs=2)
            nc.scalar.dma_start(wt_sb[:],
                                wt_d.rearrange("(kf p) f -> p kf f", p=P))
            if with_bias:
                bb = small.tile([P, D], f32, name="bb_sb", tag="bb", bufs=2)
                nc.scalar.dma_start(bb[:], bb_d[:])
            y_sb = []
            for ch in range(YCH):
                t = y_pool.tile([P, YKPC, D], bf16, name=f"y{ch}",
                                tag=f"y{ch % 4}")
                src = y_d[ch * (YKPC * P):(ch + 1) * (YKPC * P), :]
                nc.scalar.dma_start(t[:],
                                    src.rearrange("(n p) f -> p n f", p=P))
                y_sb.append(t)

            # ---- matmul 1: hT[feat, own] += y_kt^T @ slab_kt ----
            hT_ps = [psum_pool.tile([P, 512], f32, name=f"ps_{j}",
                                    tag=f"ps_{j}") for j in range(8)]
            for kt in range(KT):
                sch, si = divmod(kt, SKPC)
                ych, yi = divmod(kt, YKPC)
                for mf in range(4):
                    lhs = y_sb[ych][:, yi, mf * P:(mf + 1) * P]
                    for h in range(2):
                        nc.tensor.matmul(
                            hT_ps[mf * 2 + h], lhsT=lhs,
                            rhs=slab_sb[sch][:, si, h * 512:(h + 1) * 512],
                            start=(kt == 0), stop=(kt == KT - 1))

            # ---- evacuate hT -> bf16 SBUF [feat_part, 4, own] ----
            hT_sb = small.tile([P, 4, B], bf16, name="hT_sb", tag="hT",
                               bufs=2)
            for h in range(2):
                for mf in range(4):
                    nc.vector.tensor_copy(
                        hT_sb[:, mf, h * 512:(h + 1) * 512],
                        hT_ps[mf * 2 + h][:])

            # ---- matmul 2 + relu: out[own,:] = relu(hT^T @ W^T + b) ----
            for m in range(SCH):
                o_ps = psum_pool.tile([P, D], f32, name=f"ops_{m}",
                                      tag=f"ps_{m}")
                for kf in range(4):
                    nc.tensor.matmul(o_ps,
                                     lhsT=hT_sb[:, kf, m * P:(m + 1) * P],
                                     rhs=wt_sb[:, kf, :],
                                     start=(kf == 0), stop=(kf == 3))
                o_sb = osb_pool.tile([P, D], f32, name=f"osb{m}",
                                     tag=f"osb{m % 2}", bufs=2)
                if with_bias:
                    nc.vector.tensor_add(o_sb[:], o_ps[:], bb[:])
                    nc.vector.tensor_scalar_max(o_sb[:], o_sb[:], 0.0)
                else:
                    nc.vector.tensor_scalar_max(o_sb[:], o_ps[:], 0.0)
                nc.sync.dma_start(out_r[:, m, :], o_sb[:])

    nc.compile()
    return nc


def _prep_in_maps(x, A, W, b, with_bias):
    import ml_dtypes
    bf16 = ml_dtypes.bfloat16

    deg = A.astype(np.float32).sum(axis=1) + 1.0          # A binary, +I
    dv = (1.0 / np.sqrt(deg)).astype(np.float32)
    y = (dv[:, None] * x.astype(np.float32)).astype(bf16)
    wt = np.ascontiguousarray(W.astype(np.float32).T).astype(bf16)
    in_maps = []
    for c in range(NCORES):
        own = slice(c * B, (c + 1) * B)
        sl = np.array(A[:, own], dtype=np.float32)
        sl[np.arange(c * B, (c + 1) * B), np.arange(B)] += 1.0  # fold +I
        sl *= dv[own][None, :]                            # fold d_own^{-1/2}
        m = {"slab": sl.astype(bf16), "y": y, "wt": wt}
        if with_bias:
            m["bb"] = np.ascontiguousarray(
                np.broadcast_to(b.astype(np.float32), (P, D)))
        in_maps.append(m)
    return in_maps


def get_compiled(with_bias, ar_chunks=1, reps=1, serialize_reps=False,
                 skip_collectives=False, num_devices=NCORES, mm_n1024=False):
    key = (with_bias, ar_chunks, reps, serialize_reps, skip_collectives,
           num_devices, mm_n1024)
    if key not in _cache:
        _cache[key] = _build(with_bias, ar_chunks, reps, serialize_reps,
                             skip_collectives, num_devices, mm_n1024)
    return _cache[key]


def kernel(x, A, W, b):
    from concourse import bass_utils

    with_bias = bool(np.any(b))
    nc = get_compiled(with_bias)
    in_maps = _prep_in_maps(x, A, W, b, with_bias)
    try:
        res = bass_utils.run_bass_kernel_spmd(nc, in_maps,
                                              core_ids=list(range(NCORES)))
    except Exception:
        # the shared terminal occasionally wedges (NRT_EXEC_UNIT_UNRECOVERABLE
        # from a prior session); it auto-resets after ~1 min
        import time
        time.sleep(75)
        res = bass_utils.run_bass_kernel_spmd(nc, in_maps,
                                              core_ids=list(range(NCORES)))
    out = np.concatenate([res.results[c]["out"] for c in range(NCORES)], axis=0)
    return out.astype(np.float32)
